# revision 18
# baseline (speedup 1.0000x reference)
"""Trainium2 Bass kernel for nn_DBLayer (scatter_memory).

Semantics (matches the jax reference):
  starts[r] = flat DB start index for coordinate triple r  (prefix sums)
  gathered[r] = db[starts[r] : starts[r]+blk]              (read of ORIGINAL db)
  new_db = db, then new_db[starts[r] : starts[r]+blk] = results[r]
           applied in row order (last write wins for duplicate slots)

Layout facts hardcoded from the problem spec: db is 512 blocks x 65536 f32,
R=1024 coordinate rows, 8 cores.

Distribution (SPMD, one program, per-core data):
  - gather rows r are split contiguously: core k handles rows [128k, 128k+128)
    and needs the full db as its gather table.
  - new_db blocks are split contiguously: core k produces blocks
    [64k, 64k+64). The winning results rows for those blocks are routed to
    core k by the host and appended to the db table, so every output block is
    a *gather* from the concatenated table [db (512 blocks) | winners (64)].
  - All routing lives in an int16 index tensor (data, not code), so the same
    program runs on all 8 cores. The device moves every output byte itself:
    12 chunks per core, each = one dma_gather (128 x 32KB vectors, HBM->SBUF)
    + one contiguous 4MB write-back (SBUF->HBM), double buffered.
"""

import numpy as np

N_CORES = 8
R = 1024
P_TOTAL = 512            # blocks in db
BLK = 65536              # f32 elements per block
VEC = 8192               # f32 per gathered vector (32 KB, max dma_gather stride)
SUBS = BLK // VEC        # 8 vectors per block
ROWS_PER_CORE = R // N_CORES          # 128
SLOTS_PER_CORE = P_TOTAL // N_CORES   # 64
ROW_CHUNK = 16                        # rows per chunk -> 128 vectors
G_CHUNKS = ROWS_PER_CORE // ROW_CHUNK     # 8
S_CHUNKS = SLOTS_PER_CORE // ROW_CHUNK    # 4
N_CHUNKS = G_CHUNKS + S_CHUNKS            # 12
W_CAP = SLOTS_PER_CORE                    # winner rows routed per core
TABLE_BLOCKS = P_TOTAL + W_CAP            # 576
IDX_COLS = 128 // 16                      # idx cols per chunk (8)

_NC_CACHE = {}


def _build_nc(n_bufs=2, seq=None):
    import concourse.bacc as bacc
    import concourse.mybir as mybir
    from concourse.library_config import mlp
    from concourse._compat import get_trn_type
    from contextlib import ExitStack

    if seq is None:
        seq = list(range(N_CHUNKS))
    f32 = mybir.dt.float32
    nc = bacc.Bacc(get_trn_type() or "TRN2")
    dbw = nc.dram_tensor(
        "dbw", [TABLE_BLOCKS * SUBS, VEC], f32, kind="ExternalInput"
    )
    idx = nc.dram_tensor(
        "idx", [128, N_CHUNKS * IDX_COLS], mybir.dt.int16, kind="ExternalInput"
    )
    out = nc.dram_tensor(
        "out", [N_CHUNKS, 128, VEC], f32, kind="ExternalOutput"
    )

    with ExitStack() as stack:
        bufs = [
            stack.enter_context(
                nc.sbuf_tensor(f"buf{b}", [128, 1, VEC], f32)
            )
            for b in range(n_bufs)
        ]
        idxs = stack.enter_context(
            nc.sbuf_tensor("idxs", [128, N_CHUNKS * IDX_COLS], mybir.dt.int16)
        )
        ld = stack.enter_context(nc.semaphore("ld"))
        gs = [stack.enter_context(nc.semaphore(f"gs{b}")) for b in range(n_bufs)]
        ws = [stack.enter_context(nc.semaphore(f"ws{b}")) for b in range(n_bufs)]
        block = stack.enter_context(nc.Block())

        @block.gpsimd
        def _(g):
            g.load_library(mlp)
            g.dma_start(idxs[:], idx[:]).then_inc(ld, 16)
            g.wait_ge(ld, 16)
            for t, c in enumerate(seq):
                b = t % n_bufs
                if t >= n_bufs:
                    # buffer reusable once write-out t-n_bufs done
                    g.wait_ge(ws[b], 16 * (t // n_bufs))
                g.dma_gather(
                    bufs[b][:],
                    dbw[:],
                    idxs[:, c * IDX_COLS : (c + 1) * IDX_COLS],
                    128,
                    128,
                    VEC,
                ).then_inc(gs[b], 16)

        @block.sync
        def _(s):
            for t, c in enumerate(seq):
                b = t % n_bufs
                s.wait_ge(gs[b], 16 * (t // n_bufs + 1))
                s.dma_start(out[c], bufs[b][:, 0, :]).then_inc(ws[b], 16)
            # All write-out completions must land before nc.reset()'s
            # sem_clear, or a late +16 leaks into the next execution.
            for b in range(n_bufs):
                n_b = len([t for t in range(len(seq)) if t % n_bufs == b])
                s.wait_ge(ws[b], 16 * n_b)

    # Drain + clear semaphores so the loaded NEFF is re-runnable (PJRT keeps
    # device state across executions of the same executable).
    nc.reset()
    nc.compile()
    return nc


def _get_nc(n_bufs=2, seq=None):
    key = ("nc", n_bufs, tuple(seq) if seq is not None else None)
    if key not in _NC_CACHE:
        _NC_CACHE[key] = _build_nc(n_bufs, seq)
    return _NC_CACHE[key]


def _exclusive_prefix(x):
    return np.concatenate([np.zeros(1, x.dtype), np.cumsum(x)])


def _wrap_idx(d):
    """Host idx vector d[128] -> SBUF [128, 8] int16 layout.

    dma_gather reads idx for vector i at partition i%16, column i//16
    (replicated over the 8 partition groups for the Q7 cores)."""
    b = d.reshape(IDX_COLS, 16).T.astype(np.int16)
    return np.tile(b, (8, 1))


def _prepare_core_inputs(db2d, results, slot, win):
    """Per-core dbw/idx tensors. db2d: [512, BLK] f32 view."""
    in_maps = []
    i_sub = np.arange(128) % SUBS          # sub-vector within block
    i_row = np.arange(128) // SUBS         # row within 16-row chunk
    for k in range(N_CORES):
        # winners routed to this core, in increasing slot order
        s0 = k * SLOTS_PER_CORE
        own = np.arange(s0, s0 + SLOTS_PER_CORE)
        has_win = win[own] >= 0
        win_rows = win[own][has_win]
        winners = np.zeros((W_CAP, BLK), np.float32)
        if len(win_rows):
            winners[: len(win_rows)] = results[win_rows]
        dbw = np.concatenate([db2d, winners], 0).reshape(TABLE_BLOCKS * SUBS, VEC)

        # per-output-slot source block in the concatenated table
        srcblk = own.copy()
        srcblk[has_win] = P_TOTAL + np.arange(len(win_rows))

        cols = []
        for c in range(G_CHUNKS):
            rows = k * ROWS_PER_CORE + c * ROW_CHUNK + i_row
            d = slot[rows] * SUBS + i_sub
            cols.append(_wrap_idx(d))
        for c in range(S_CHUNKS):
            t = c * ROW_CHUNK + i_row
            d = srcblk[t] * SUBS + i_sub
            cols.append(_wrap_idx(d))
        idx = np.concatenate(cols, 1)
        in_maps.append({"dbw": dbw, "idx": idx})
    return in_maps


def _compute_routing(db, results, fields, params_per_field, scaled_params, coords):
    pf = _exclusive_prefix(fields.astype(np.int64))
    pp = _exclusive_prefix(params_per_field.astype(np.int64))
    ps = _exclusive_prefix(scaled_params.astype(np.int64))
    abs_field = pf[coords[:, 0]] + coords[:, 1]
    abs_param = pp[abs_field] + coords[:, 2]
    starts = ps[abs_param]
    blk = results.shape[1]
    assert blk == BLK and len(starts) == R and db.size == P_TOTAL * BLK
    assert (starts % BLK == 0).all() and (starts < P_TOTAL * BLK).all()
    slot = (starts // BLK).astype(np.int64)
    win = -np.ones(P_TOTAL, np.int64)
    win[slot] = np.arange(R)  # numpy fancy assign: last write wins
    return slot, win


def _get_runner(n_bufs=2, seq=None):
    """Cached jit(shard_map(bass_exec)) runner over the 8 cores.

    Modeled on concourse.bass2jax.run_bass_via_pjrt, but built once and
    reused: repeat calls skip re-lowering, and callers can pass
    device-resident arrays to skip host transfers."""
    key = ("runner", n_bufs, tuple(seq) if seq is not None else None)
    if key in _NC_CACHE:
        return _NC_CACHE[key]
    import jax
    import concourse.mybir as mybir
    from concourse.bass2jax import (
        _bass_exec_p,
        install_neuronx_cc_hook,
        partition_id_tensor,
    )
    from jax.experimental.shard_map import shard_map
    from jax.sharding import Mesh, PartitionSpec

    install_neuronx_cc_hook()
    nc = _get_nc(n_bufs, seq)
    partition_name = (
        nc.partition_id_tensor.name if nc.partition_id_tensor else None
    )

    in_names, out_names, out_avals, zero_outs = [], [], [], []
    for alloc in nc.m.functions[0].allocations:
        if not isinstance(alloc, mybir.MemoryLocationSet):
            continue
        name = alloc.memorylocations[0].name
        if alloc.kind == "ExternalInput":
            if name != partition_name:
                in_names.append(name)
        elif alloc.kind == "ExternalOutput":
            shape = tuple(alloc.tensor_shape)
            dtype = mybir.dt.np(alloc.dtype)
            out_names.append(name)
            out_avals.append(jax.core.ShapedArray(shape, dtype))
            zero_outs.append(np.zeros(shape, dtype))
    n_params = len(in_names)
    n_outs = len(out_avals)
    all_names = in_names + out_names
    if partition_name is not None:
        all_names = all_names + [partition_name]
    donate = tuple(range(n_params, n_params + n_outs))

    def _body(*args):
        operands = list(args)
        if partition_name is not None:
            operands.append(partition_id_tensor())
        outs = _bass_exec_p.bind(
            *operands,
            out_avals=tuple(out_avals),
            in_names=tuple(all_names),
            out_names=tuple(out_names),
            lowering_input_output_aliases=(),
            sim_require_finite=True,
            sim_require_nnan=True,
            nc=nc,
        )
        return tuple(outs)

    devices = jax.devices()[:N_CORES]
    mesh = Mesh(np.asarray(devices), ("core",))
    sharded = jax.jit(
        shard_map(
            _body,
            mesh=mesh,
            in_specs=(PartitionSpec("core"),) * (n_params + n_outs),
            out_specs=(PartitionSpec("core"),) * n_outs,
            check_rep=False,
        ),
        donate_argnums=donate,
        keep_unused=True,
    )
    runner = {
        "fn": sharded,
        "in_names": in_names,
        "out_names": out_names,
        "out_avals": out_avals,
        "zero_outs": zero_outs,
        "mesh": mesh,
    }
    _NC_CACHE[key] = runner
    return runner


N_BUFS_DEFAULT = 4


def _run_spmd(in_maps, trace=False):
    r = _get_runner(N_BUFS_DEFAULT)
    concat_in = [
        np.concatenate([np.asarray(m[name]) for m in in_maps], axis=0)
        for name in r["in_names"]
    ]
    concat_zeros = [
        np.zeros((N_CORES * z.shape[0], *z.shape[1:]), z.dtype)
        for z in r["zero_outs"]
    ]
    out_arrs = r["fn"](*concat_in, *concat_zeros)

    class _Res:
        pass

    res = _Res()
    res.results = [
        {
            name: np.asarray(out_arrs[i]).reshape(
                N_CORES, *r["out_avals"][i].shape
            )[c]
            for i, name in enumerate(r["out_names"])
        }
        for c in range(N_CORES)
    ]
    res.exec_time_ns = None
    res.mean_exec_time_ns = None
    res.max_exec_time_core_id = None
    return res


def measure_exec_ns(in_maps, iters=30, warmup=3, n_bufs=2, seq=None):
    """Amortized per-execution wall time with device-resident inputs.

    Inputs are device_put once; each iteration donates the previous
    iteration's outputs as the next execution's output operands, so the
    steady-state loop moves no host data. Returns (avg_ns, best_ns, out_arrs).
    """
    import time as _time
    import jax
    from jax.sharding import NamedSharding, PartitionSpec

    r = _get_runner(n_bufs, seq)
    sh = NamedSharding(r["mesh"], PartitionSpec("core"))
    concat_in = [
        np.concatenate([np.asarray(m[name]) for m in in_maps], axis=0)
        for name in r["in_names"]
    ]
    dev_in = [jax.device_put(a, sh) for a in concat_in]
    outs = [
        jax.device_put(
            np.zeros((N_CORES * z.shape[0], *z.shape[1:]), z.dtype), sh
        )
        for z in r["zero_outs"]
    ]
    for _ in range(warmup):
        outs = list(r["fn"](*dev_in, *outs))
    jax.block_until_ready(outs)
    times = []
    for _ in range(iters):
        t0 = _time.perf_counter()
        outs = list(r["fn"](*dev_in, *outs))
        jax.block_until_ready(outs)
        times.append((_time.perf_counter() - t0) * 1e9)
    # pipelined rate (dispatch overlapped)
    jax.block_until_ready(outs)
    t0 = _time.perf_counter()
    for _ in range(iters):
        outs = list(r["fn"](*dev_in, *outs))
    jax.block_until_ready(outs)
    piped = (_time.perf_counter() - t0) * 1e9 / iters
    return {
        "avg_ns": float(np.mean(times)),
        "best_ns": float(np.min(times)),
        "piped_ns": piped,
        "outs": outs,
    }


def kernel(db, results, fields, params_per_field, scaled_params, coords,
           _trace=False):
    db = np.ascontiguousarray(np.asarray(db), dtype=np.float32)
    results = np.ascontiguousarray(np.asarray(results), dtype=np.float32)
    fields = np.asarray(fields)
    params_per_field = np.asarray(params_per_field)
    scaled_params = np.asarray(scaled_params)
    coords = np.asarray(coords)

    slot, win = _compute_routing(
        db, results, fields, params_per_field, scaled_params, coords
    )
    db2d = db.reshape(P_TOTAL, BLK)
    in_maps = _prepare_core_inputs(db2d, results, slot, win)

    res = _run_spmd(in_maps, trace=_trace)
    outs = [np.asarray(res.results[k]["out"]) for k in range(N_CORES)]

    gathered = np.concatenate(
        [o[:G_CHUNKS].reshape(ROWS_PER_CORE, BLK) for o in outs], 0
    )
    new_db = np.concatenate(
        [o[G_CHUNKS:].reshape(SLOTS_PER_CORE * BLK) for o in outs], 0
    )
    if _trace:
        return (gathered, new_db), res
    return gathered, new_db
